# revision 30
# baseline (speedup 1.0000x reference)
"""Trainium2 Bass kernel for nn_BsplineLoss (chamfer between skeletal points
and bspline curve points).

Full-input contract: kernel(**inputs) takes the unsharded arrays
  skeletal_points      (16, 4096, 3) f32
  primitive_parameters (16, 64, 12)  f32
  bspline_basis        (16, 4)       f32
and returns the full (16,) f32 loss.

Sharding: data-parallel over batch B=16 across 8 cores (2 batches/core).

Host prep (marshalling): build the split-precision matmul operands
  lh6 (13, NB, P, NCHUNK) bf16: rows 0-2 a_hi, 3-5 a_lo, 6-8 a_hi,
                                9-10 ones, 11-12 a2_hi/lo
  rhs (13, NB*M) bf16:          rows 0-5 R0=bf16(2b), 6-8 R1=2b-R0,
                                9-10 (-|b|^2)_hi/lo, 11-12 -1
so that matmul(lh6[:,b,:,j]^T @ rhs[:,b]) = 2 a.b - |b|^2 - |a|^2 = -d2.

Device main loop (per batch, 16 chunk-pairs):
  4 matmuls -> psum (128, 2048) = -d2 for 2 chunks
  ScalarE drain: sbd = -psum (bf16, +d2)
  DVE rowfold (per chunk): custom dual-stream min-accum op on chunk halves
    -> rowmin_d2 (tensor_tensor_reduce compiles but dies at runtime on this
    stack; GpSimd/Pool has no elementwise min - compiler engine check)
  even pairs: DVE pairfold pf = min(chunk0, chunk1) -> DMA 256KB
  odd pairs:  DMA raw sbd 512KB (DVE has no headroom for every pairfold;
    a full raw dump is DMA-completion-bound at ~3.6us per 512KB transfer)
Host: relu/sqrt/mean over rows; min over (pairs, chunks, partitions) +
relu/sqrt/mean for cols.
"""

import numpy as np

P = 128
NB = 2          # batches per core
NCHUNK = 32     # p-chunks per batch (chunk j = points {32r + j})
NPAIR = NCHUNK // 2
JPP = 32        # points per partition per batch
M = 1024        # curve points per batch
NCORES = 8

_CACHE = {}


def _register_min_op():
    """Custom DVE op: out = min(in0, in1); accum_out = min(c0, min_k out).
    Reads two SBUF streams at 1 elem/cycle/lane each."""
    from concourse import dve_ops
    from concourse.dve_spec import Spec, minn, Src0, Src1, C0, lower, _has_src1
    from concourse.dve_uop import DveOpSpec
    import numpy as np

    name = "TT_MIN_RED_ANT"
    for o in dve_ops.OPS:
        if o.name == name:
            return o

    def _ref(in0, in1, c0, c1, c2):
        body = np.minimum(in0.astype(np.float32), in1.astype(np.float32))
        acc = np.minimum(
            c0, body.reshape(body.shape[0], -1).min(axis=-1, keepdims=True)
        )
        return body, acc

    spec = Spec(body=minn(Src0, Src1), accum=minn, accum_init=C0, reference=_ref)
    opcode = max(dve_ops._SUB_OPCODE_FOR_NAME.values()) + 1
    assert opcode < 0x20
    shas = {}
    for ver in ("v3", "v4"):
        try:
            s = DveOpSpec(
                name=name, opcode=opcode, uops=lower(spec, ver=ver),
                rd1_en=_has_src1(spec),
            )
            shas[ver] = s.sha(ver)
        except Exception:
            pass
    op = dve_ops.DveOp(name, spec, subdim=False, uops_sha=shas,
                       perf_en={"v3": True, "v4": True})
    dve_ops.OPS.append(op)
    dve_ops.CUSTOM_DVE_SPECS[name] = spec
    dve_ops._SUB_OPCODE_FOR_NAME[name] = opcode
    return op


def _build_nc():
    import concourse.bacc as bacc
    import concourse.tile as tile
    from concourse import mybir

    f32 = mybir.dt.float32
    bf16 = mybir.dt.bfloat16
    AL = mybir.AluOpType

    min_op = _register_min_op()
    nc = bacc.Bacc(None, target_bir_lowering=False)

    lh6d = nc.dram_tensor("lh6", [13, NCHUNK * NB * P], bf16, kind="ExternalInput")
    rhsd = nc.dram_tensor("rhs", [13, NB * M], bf16, kind="ExternalInput")

    orow = nc.dram_tensor("orow", [P, NB * NCHUNK], f32, kind="ExternalOutput")
    oc2p = nc.dram_tensor(
        "oc2p", [NB, NPAIR // 2, P, M], bf16, kind="ExternalOutput"
    )
    oc2r = nc.dram_tensor(
        "oc2r", [NB, NPAIR // 2, P, 2 * M], bf16, kind="ExternalOutput"
    )

    with tile.TileContext(nc) as tc:
        with (
            tc.tile_pool(name="persist", bufs=1) as persist,
            tc.tile_pool(name="mpsum", bufs=2, space="PSUM") as mpsum,
            tc.tile_pool(name="sbp", bufs=3) as sbp,
            tc.tile_pool(name="pfp", bufs=2) as pfp,
        ):
            lh6 = persist.tile([13, NCHUNK, NB, P], bf16)
            rhs = persist.tile([13, NB * M], bf16)
            rowraw = persist.tile([P, NB * NCHUNK], f32)
            junk = persist.tile([P, 512], bf16)

            # split the weight load so chunk 0/1 weights land fast and the
            # first matmuls don't wait on the full 208KB transfer
            lh6v = lh6[:].rearrange("k j b r -> k (j b r)")
            cut0 = 2 * NB * P
            nc.sync.dma_start(lh6v[:, 0:cut0], lh6d[:, 0:cut0])
            nc.scalar.dma_start(lh6v[:, cut0:], lh6d[:, cut0:])
            nc.gpsimd.dma_start(rhs[:], rhsd[:])

            for b in range(NB):
                for pr in range(NPAIR):
                    ps = mpsum.tile([P, 2 * M], f32, tag="ps")
                    for u in range(2):
                        j = 2 * pr + u
                        lhsT = lh6[:, j, b, :]
                        for h in range(2):
                            nc.tensor.matmul(
                                ps[:, u * M + h * 512 : u * M + (h + 1) * 512],
                                lhsT,
                                rhs[:, b * M + h * 512 : b * M + (h + 1) * 512],
                            )
                    sbd = sbp.tile([P, 2 * M], bf16, tag="sbd")
                    nc.scalar.mul(sbd[:], ps[:], -1.0)  # +d2, bf16
                    for u in range(2):
                        col = b * NCHUNK + 2 * pr + u
                        nc.vector._custom_dve(
                            min_op,
                            out=junk[:],
                            in0=sbd[:, u * M : u * M + 512],
                            in1=sbd[:, u * M + 512 : (u + 1) * M],
                            s0=3.0e38,
                            accum_out=rowraw[:, col : col + 1],
                        )
                    if pr % 2 == 1:
                        pf = pfp.tile([P, M], bf16, tag="pf")
                        nc.vector.tensor_tensor(
                            out=pf[:], in0=sbd[:, 0:M], in1=sbd[:, M : 2 * M],
                            op=AL.min,
                        )
                        nc.sync.dma_start(oc2p[b, pr // 2], pf[:])
                    else:
                        nc.gpsimd.dma_start(oc2r[b, pr // 2], sbd[:])
                nc.sync.dma_start(
                    orow[:, b * NCHUNK : (b + 1) * NCHUNK],
                    rowraw[:, b * NCHUNK : (b + 1) * NCHUNK],
                )

    nc.compile()
    return nc


def _get_nc():
    if "nc" not in _CACHE:
        _CACHE["nc"] = _build_nc()
    return _CACHE["nc"]


def _split_bf16(x):
    import ml_dtypes

    hi = x.astype(ml_dtypes.bfloat16)
    lo = (x - hi.astype(np.float32)).astype(ml_dtypes.bfloat16)
    return hi, lo


def make_in_maps(skeletal_points, primitive_parameters, bspline_basis):
    import ml_dtypes

    bf16 = ml_dtypes.bfloat16
    skel = np.ascontiguousarray(skeletal_points, dtype=np.float32)
    prim = np.ascontiguousarray(primitive_parameters, dtype=np.float32)
    basis = np.ascontiguousarray(bspline_basis, dtype=np.float32)

    in_maps = []
    for c in range(NCORES):
        lh6 = np.empty((13, NCHUNK, NB, P), dtype=bf16)
        rhs = np.empty((13, NB * M), dtype=bf16)
        for b in range(NB):
            pts = skel[NB * c + b].reshape(P, JPP, 3)  # point p = 32r + j
            a_hi, a_lo = _split_bf16(pts)
            a2 = (pts.astype(np.float64) ** 2).sum(-1).astype(np.float32)
            a2_hi, a2_lo = _split_bf16(a2)
            for cc in range(3):
                lh6[0 + cc, :, b, :] = a_hi[:, :, cc].T
                lh6[3 + cc, :, b, :] = a_lo[:, :, cc].T
                lh6[6 + cc, :, b, :] = a_hi[:, :, cc].T
            lh6[9, :, b, :] = np.float32(1.0)
            lh6[10, :, b, :] = np.float32(1.0)
            lh6[11, :, b, :] = a2_hi.T
            lh6[12, :, b, :] = a2_lo.T

            ctrl = prim[NB * c + b].reshape(64, 4, 3)
            curves = np.einsum("tn,knc->ktc", basis, ctrl).reshape(M, 3)
            b2 = (curves.astype(np.float64) ** 2).sum(-1).astype(np.float32)
            r0, r1 = _split_bf16(2.0 * curves)
            nb2h, nb2l = _split_bf16(-b2)
            sl = slice(b * M, (b + 1) * M)
            for cc in range(3):
                rhs[0 + cc, sl] = r0[:, cc]
                rhs[3 + cc, sl] = r0[:, cc]
                rhs[6 + cc, sl] = r1[:, cc]
            rhs[9, sl] = nb2h
            rhs[10, sl] = nb2l
            rhs[11, sl] = np.float32(-1.0)
            rhs[12, sl] = np.float32(-1.0)
        in_maps.append({"lh6": lh6.reshape(13, NCHUNK * NB * P), "rhs": rhs})
    return in_maps


def postprocess(results):
    """results: list of 8 per-core dicts with orow/oc2."""
    import ml_dtypes

    loss = np.zeros(16, dtype=np.float32)
    for c, r in enumerate(results):
        rowd2 = np.maximum(np.asarray(r["orow"]).astype(np.float64), 0.0)  # (128, 64)
        # +d2 dumps. Fold as uint16: the bf16 bit pattern is monotonic for
        # values >= 0, and rare tiny negatives (psum rounding) sort above
        # everything, matching the relu semantics.
        ocp = (
            np.ascontiguousarray(np.asarray(r["oc2p"]))
            .view(np.uint16)
            .reshape(NB, NPAIR // 2, P, M)
        )
        ocr = (
            np.ascontiguousarray(np.asarray(r["oc2r"]))
            .view(np.uint16)
            .reshape(NB, NPAIR // 2, P, 2, M)
        )
        for b in range(NB):
            cha = np.sqrt(rowd2[:, b * NCHUNK : (b + 1) * NCHUNK]).mean()
            cu = np.minimum(
                ocp[b].min(axis=(0, 1)), ocr[b].min(axis=(0, 1, 2))
            )
            cold2 = cu.view(ml_dtypes.bfloat16).astype(np.float64)
            chb = np.sqrt(np.maximum(cold2, 0.0)).mean()
            loss[NB * c + b] = np.float32(cha + chb)
    return loss


def kernel(skeletal_points, primitive_parameters, bspline_basis):
    from concourse.bass_utils import run_bass_kernel_spmd

    nc = _get_nc()
    in_maps = make_in_maps(skeletal_points, primitive_parameters, bspline_basis)
    res = run_bass_kernel_spmd(nc, in_maps, core_ids=list(range(NCORES)))
    return postprocess(res.results)


# revision 31
# speedup vs baseline: 1.0204x; 1.0204x over previous
"""Trainium2 Bass kernel for nn_BsplineLoss (chamfer between skeletal points
and bspline curve points).

Full-input contract: kernel(**inputs) takes the unsharded arrays
  skeletal_points      (16, 4096, 3) f32
  primitive_parameters (16, 64, 12)  f32
  bspline_basis        (16, 4)       f32
and returns the full (16,) f32 loss.

Sharding: data-parallel over batch B=16 across 8 cores (2 batches/core).

Host prep (marshalling): build the split-precision matmul operands
  lh6 (13, NB, P, NCHUNK) bf16: rows 0-2 a_hi, 3-5 a_lo, 6-8 a_hi,
                                9-10 ones, 11-12 a2_hi/lo
  rhs (13, NB*M) bf16:          rows 0-5 R0=bf16(2b), 6-8 R1=2b-R0,
                                9-10 (-|b|^2)_hi/lo, 11-12 -1
so that matmul(lh6[:,b,:,j]^T @ rhs[:,b]) = 2 a.b - |b|^2 - |a|^2 = -d2.

Device main loop (per batch, 16 chunk-pairs):
  4 matmuls -> psum (128, 2048) = -d2 for 2 chunks
  ScalarE drain: sbd = -psum (bf16, +d2)
  DVE rowfold (per chunk): custom dual-stream min-accum op on chunk halves
    -> rowmin_d2 (tensor_tensor_reduce compiles but dies at runtime on this
    stack; GpSimd/Pool has no elementwise min - compiler engine check)
  even pairs: DVE pairfold pf = min(chunk0, chunk1) -> DMA 256KB
  odd pairs:  DMA raw sbd 512KB (DVE has no headroom for every pairfold;
    a full raw dump is DMA-completion-bound at ~3.6us per 512KB transfer)
Host: relu/sqrt/mean over rows; min over (pairs, chunks, partitions) +
relu/sqrt/mean for cols.
"""

import numpy as np

P = 128
NB = 2          # batches per core
NCHUNK = 32     # p-chunks per batch (chunk j = points {32r + j})
NPAIR = NCHUNK // 2
JPP = 32        # points per partition per batch
M = 1024        # curve points per batch
NCORES = 8

_CACHE = {}


def _register_min_op():
    """Custom DVE op: out = min(in0, in1); accum_out = min(c0, min_k out).
    Reads two SBUF streams at 1 elem/cycle/lane each."""
    from concourse import dve_ops
    from concourse.dve_spec import Spec, minn, Src0, Src1, C0, lower, _has_src1
    from concourse.dve_uop import DveOpSpec
    import numpy as np

    name = "TT_MIN_RED_ANT"
    for o in dve_ops.OPS:
        if o.name == name:
            return o

    def _ref(in0, in1, c0, c1, c2):
        body = np.minimum(in0.astype(np.float32), in1.astype(np.float32))
        acc = np.minimum(
            c0, body.reshape(body.shape[0], -1).min(axis=-1, keepdims=True)
        )
        return body, acc

    spec = Spec(body=minn(Src0, Src1), accum=minn, accum_init=C0, reference=_ref)
    opcode = max(dve_ops._SUB_OPCODE_FOR_NAME.values()) + 1
    assert opcode < 0x20
    shas = {}
    for ver in ("v3", "v4"):
        try:
            s = DveOpSpec(
                name=name, opcode=opcode, uops=lower(spec, ver=ver),
                rd1_en=_has_src1(spec),
            )
            shas[ver] = s.sha(ver)
        except Exception:
            pass
    op = dve_ops.DveOp(name, spec, subdim=False, uops_sha=shas,
                       perf_en={"v3": True, "v4": True})
    dve_ops.OPS.append(op)
    dve_ops.CUSTOM_DVE_SPECS[name] = spec
    dve_ops._SUB_OPCODE_FOR_NAME[name] = opcode
    return op


def _build_nc():
    import concourse.bacc as bacc
    import concourse.tile as tile
    from concourse import mybir

    f32 = mybir.dt.float32
    bf16 = mybir.dt.bfloat16
    AL = mybir.AluOpType

    min_op = _register_min_op()
    nc = bacc.Bacc(None, target_bir_lowering=False)

    lh6d = nc.dram_tensor("lh6", [13, NCHUNK * NB * P], bf16, kind="ExternalInput")
    rhsd = nc.dram_tensor("rhs", [13, NB * M], bf16, kind="ExternalInput")

    orow = nc.dram_tensor("orow", [P, NB * NCHUNK], f32, kind="ExternalOutput")
    oc2p = nc.dram_tensor(
        "oc2p", [NB, NPAIR // 2, P, M], bf16, kind="ExternalOutput"
    )
    oc2r = nc.dram_tensor(
        "oc2r", [NB, NPAIR // 2, P, 2 * M], bf16, kind="ExternalOutput"
    )

    with tile.TileContext(nc) as tc:
        with (
            tc.tile_pool(name="persist", bufs=1) as persist,
            tc.tile_pool(name="mpsum", bufs=2, space="PSUM") as mpsum,
            tc.tile_pool(name="sbp", bufs=3) as sbp,
            tc.tile_pool(name="pfp", bufs=2) as pfp,
        ):
            lh6 = persist.tile([13, NCHUNK, NB, P], bf16)
            rhs = persist.tile([13, NB * M], bf16)
            rowraw = persist.tile([P, NB * NCHUNK], f32)
            junk = persist.tile([P, 512], bf16)

            # split the weight load so chunk 0/1 weights land fast and the
            # first matmuls don't wait on the full 208KB transfer
            lh6v = lh6[:].rearrange("k j b r -> k (j b r)")
            cut0 = 2 * NB * P
            nc.sync.dma_start(lh6v[:, 0:cut0], lh6d[:, 0:cut0])
            nc.scalar.dma_start(lh6v[:, cut0:], lh6d[:, cut0:])
            nc.gpsimd.dma_start(rhs[:], rhsd[:])

            for b in range(NB):
                for pr in range(NPAIR):
                    ps = mpsum.tile([P, 2 * M], f32, tag="ps")
                    for u in range(2):
                        j = 2 * pr + u
                        lhsT = lh6[:, j, b, :]
                        for h in range(2):
                            nc.tensor.matmul(
                                ps[:, u * M + h * 512 : u * M + (h + 1) * 512],
                                lhsT,
                                rhs[:, b * M + h * 512 : b * M + (h + 1) * 512],
                            )
                    sbd = sbp.tile([P, 2 * M], bf16, tag="sbd")
                    nc.scalar.mul(sbd[:], ps[:], -1.0)  # +d2, bf16
                    for u in range(2):
                        col = b * NCHUNK + 2 * pr + u
                        nc.vector._custom_dve(
                            min_op,
                            out=junk[:],
                            in0=sbd[:, u * M : u * M + 512],
                            in1=sbd[:, u * M + 512 : (u + 1) * M],
                            s0=3.0e38,
                            accum_out=rowraw[:, col : col + 1],
                        )
                    if pr % 2 == 0:
                        pf = pfp.tile([P, M], bf16, tag="pf")
                        nc.vector.tensor_tensor(
                            out=pf[:], in0=sbd[:, 0:M], in1=sbd[:, M : 2 * M],
                            op=AL.min,
                        )
                        nc.sync.dma_start(oc2p[b, pr // 2], pf[:])
                    else:
                        nc.gpsimd.dma_start(oc2r[b, pr // 2], sbd[:])
                nc.sync.dma_start(
                    orow[:, b * NCHUNK : (b + 1) * NCHUNK],
                    rowraw[:, b * NCHUNK : (b + 1) * NCHUNK],
                )

    nc.compile()
    return nc


def _get_nc():
    if "nc" not in _CACHE:
        _CACHE["nc"] = _build_nc()
    return _CACHE["nc"]


def _split_bf16(x):
    import ml_dtypes

    hi = x.astype(ml_dtypes.bfloat16)
    lo = (x - hi.astype(np.float32)).astype(ml_dtypes.bfloat16)
    return hi, lo


def make_in_maps(skeletal_points, primitive_parameters, bspline_basis):
    import ml_dtypes

    bf16 = ml_dtypes.bfloat16
    skel = np.ascontiguousarray(skeletal_points, dtype=np.float32)
    prim = np.ascontiguousarray(primitive_parameters, dtype=np.float32)
    basis = np.ascontiguousarray(bspline_basis, dtype=np.float32)

    in_maps = []
    for c in range(NCORES):
        lh6 = np.empty((13, NCHUNK, NB, P), dtype=bf16)
        rhs = np.empty((13, NB * M), dtype=bf16)
        for b in range(NB):
            pts = skel[NB * c + b].reshape(P, JPP, 3)  # point p = 32r + j
            a_hi, a_lo = _split_bf16(pts)
            a2 = (pts.astype(np.float64) ** 2).sum(-1).astype(np.float32)
            a2_hi, a2_lo = _split_bf16(a2)
            for cc in range(3):
                lh6[0 + cc, :, b, :] = a_hi[:, :, cc].T
                lh6[3 + cc, :, b, :] = a_lo[:, :, cc].T
                lh6[6 + cc, :, b, :] = a_hi[:, :, cc].T
            lh6[9, :, b, :] = np.float32(1.0)
            lh6[10, :, b, :] = np.float32(1.0)
            lh6[11, :, b, :] = a2_hi.T
            lh6[12, :, b, :] = a2_lo.T

            ctrl = prim[NB * c + b].reshape(64, 4, 3)
            curves = np.einsum("tn,knc->ktc", basis, ctrl).reshape(M, 3)
            b2 = (curves.astype(np.float64) ** 2).sum(-1).astype(np.float32)
            r0, r1 = _split_bf16(2.0 * curves)
            nb2h, nb2l = _split_bf16(-b2)
            sl = slice(b * M, (b + 1) * M)
            for cc in range(3):
                rhs[0 + cc, sl] = r0[:, cc]
                rhs[3 + cc, sl] = r0[:, cc]
                rhs[6 + cc, sl] = r1[:, cc]
            rhs[9, sl] = nb2h
            rhs[10, sl] = nb2l
            rhs[11, sl] = np.float32(-1.0)
            rhs[12, sl] = np.float32(-1.0)
        in_maps.append({"lh6": lh6.reshape(13, NCHUNK * NB * P), "rhs": rhs})
    return in_maps


def postprocess(results):
    """results: list of 8 per-core dicts with orow/oc2."""
    import ml_dtypes

    loss = np.zeros(16, dtype=np.float32)
    for c, r in enumerate(results):
        rowd2 = np.maximum(np.asarray(r["orow"]).astype(np.float64), 0.0)  # (128, 64)
        # +d2 dumps. Fold as uint16: the bf16 bit pattern is monotonic for
        # values >= 0, and rare tiny negatives (psum rounding) sort above
        # everything, matching the relu semantics.
        ocp = (
            np.ascontiguousarray(np.asarray(r["oc2p"]))
            .view(np.uint16)
            .reshape(NB, NPAIR // 2, P, M)
        )
        ocr = (
            np.ascontiguousarray(np.asarray(r["oc2r"]))
            .view(np.uint16)
            .reshape(NB, NPAIR // 2, P, 2, M)
        )
        for b in range(NB):
            cha = np.sqrt(rowd2[:, b * NCHUNK : (b + 1) * NCHUNK]).mean()
            cu = np.minimum(
                ocp[b].min(axis=(0, 1)), ocr[b].min(axis=(0, 1, 2))
            )
            cold2 = cu.view(ml_dtypes.bfloat16).astype(np.float64)
            chb = np.sqrt(np.maximum(cold2, 0.0)).mean()
            loss[NB * c + b] = np.float32(cha + chb)
    return loss


def kernel(skeletal_points, primitive_parameters, bspline_basis):
    from concourse.bass_utils import run_bass_kernel_spmd

    nc = _get_nc()
    in_maps = make_in_maps(skeletal_points, primitive_parameters, bspline_basis)
    res = run_bass_kernel_spmd(nc, in_maps, core_ids=list(range(NCORES)))
    return postprocess(res.results)


# revision 32
# speedup vs baseline: 1.0368x; 1.0161x over previous
"""Trainium2 Bass kernel for nn_BsplineLoss (chamfer between skeletal points
and bspline curve points).

Full-input contract: kernel(**inputs) takes the unsharded arrays
  skeletal_points      (16, 4096, 3) f32
  primitive_parameters (16, 64, 12)  f32
  bspline_basis        (16, 4)       f32
and returns the full (16,) f32 loss.

Sharding: data-parallel over batch B=16 across 8 cores (2 batches/core).

Host prep (marshalling): build the split-precision matmul operands
  lh6 (13, NB, P, NCHUNK) bf16: rows 0-2 a_hi, 3-5 a_lo, 6-8 a_hi,
                                9-10 ones, 11-12 a2_hi/lo
  rhs (13, NB*M) bf16:          rows 0-5 R0=bf16(2b), 6-8 R1=2b-R0,
                                9-10 (-|b|^2)_hi/lo, 11-12 -1
so that matmul(lh6[:,b,:,j]^T @ rhs[:,b]) = 2 a.b - |b|^2 - |a|^2 = -d2.

Device main loop (per batch, 16 chunk-pairs):
  4 matmuls -> psum (128, 2048) = -d2 for 2 chunks
  ScalarE drain: sbd = -psum (bf16, +d2)
  DVE rowfold (per chunk): custom dual-stream min-accum op on chunk halves
    -> rowmin_d2 (tensor_tensor_reduce compiles but dies at runtime on this
    stack; GpSimd/Pool has no elementwise min - compiler engine check)
  even pairs: DVE pairfold pf = min(chunk0, chunk1) -> DMA 256KB
  odd pairs:  DMA raw sbd 512KB (DVE has no headroom for every pairfold;
    a full raw dump is DMA-completion-bound at ~3.6us per 512KB transfer)
Host: relu/sqrt/mean over rows; min over (pairs, chunks, partitions) +
relu/sqrt/mean for cols.
"""

import numpy as np

P = 128
NB = 2          # batches per core
NCHUNK = 32     # p-chunks per batch (chunk j = points {32r + j})
NPAIR = NCHUNK // 2
JPP = 32        # points per partition per batch
M = 1024        # curve points per batch
NCORES = 8

_CACHE = {}


def _register_min_op():
    """Custom DVE op: out = min(in0, in1); accum_out = min(c0, min_k out).
    Reads two SBUF streams at 1 elem/cycle/lane each."""
    from concourse import dve_ops
    from concourse.dve_spec import Spec, minn, Src0, Src1, C0, lower, _has_src1
    from concourse.dve_uop import DveOpSpec
    import numpy as np

    name = "TT_MIN_RED_ANT"
    for o in dve_ops.OPS:
        if o.name == name:
            return o

    def _ref(in0, in1, c0, c1, c2):
        body = np.minimum(in0.astype(np.float32), in1.astype(np.float32))
        acc = np.minimum(
            c0, body.reshape(body.shape[0], -1).min(axis=-1, keepdims=True)
        )
        return body, acc

    spec = Spec(body=minn(Src0, Src1), accum=minn, accum_init=C0, reference=_ref)
    opcode = max(dve_ops._SUB_OPCODE_FOR_NAME.values()) + 1
    assert opcode < 0x20
    shas = {}
    for ver in ("v3", "v4"):
        try:
            s = DveOpSpec(
                name=name, opcode=opcode, uops=lower(spec, ver=ver),
                rd1_en=_has_src1(spec),
            )
            shas[ver] = s.sha(ver)
        except Exception:
            pass
    op = dve_ops.DveOp(name, spec, subdim=False, uops_sha=shas,
                       perf_en={"v3": True, "v4": True})
    dve_ops.OPS.append(op)
    dve_ops.CUSTOM_DVE_SPECS[name] = spec
    dve_ops._SUB_OPCODE_FOR_NAME[name] = opcode
    return op


def _build_nc():
    import concourse.bacc as bacc
    import concourse.tile as tile
    from concourse import mybir

    f32 = mybir.dt.float32
    bf16 = mybir.dt.bfloat16
    AL = mybir.AluOpType

    min_op = _register_min_op()
    nc = bacc.Bacc(None, target_bir_lowering=False)

    lh6d = nc.dram_tensor("lh6", [13, NCHUNK * NB * P], bf16, kind="ExternalInput")
    rhsd = nc.dram_tensor("rhs", [13, NB * M], bf16, kind="ExternalInput")

    orow = nc.dram_tensor("orow", [P, NB * NCHUNK], f32, kind="ExternalOutput")
    oc2p = nc.dram_tensor(
        "oc2p", [NB, NPAIR // 2, P, M], bf16, kind="ExternalOutput"
    )
    oc2r = nc.dram_tensor(
        "oc2r", [NB, NPAIR // 2, P, 2 * M], bf16, kind="ExternalOutput"
    )

    with tile.TileContext(nc) as tc:
        with (
            tc.tile_pool(name="persist", bufs=1) as persist,
            tc.tile_pool(name="mpsum", bufs=2, space="PSUM") as mpsum,
            tc.tile_pool(name="sbp", bufs=4) as sbp,
            tc.tile_pool(name="pfp", bufs=3) as pfp,
        ):
            lh6 = persist.tile([13, NCHUNK, NB, P], bf16)
            rhs = persist.tile([13, NB * M], bf16)
            rowraw = persist.tile([P, NB * NCHUNK], f32)
            junk = persist.tile([P, 512], bf16)

            # split the weight load so chunk 0/1 weights land fast and the
            # first matmuls don't wait on the full 208KB transfer
            lh6v = lh6[:].rearrange("k j b r -> k (j b r)")
            cut0 = 2 * NB * P
            nc.sync.dma_start(lh6v[:, 0:cut0], lh6d[:, 0:cut0])
            nc.scalar.dma_start(lh6v[:, cut0:], lh6d[:, cut0:])
            nc.gpsimd.dma_start(rhs[:], rhsd[:])

            for b in range(NB):
                for pr in range(NPAIR):
                    ps = mpsum.tile([P, 2 * M], f32, tag="ps")
                    for u in range(2):
                        j = 2 * pr + u
                        lhsT = lh6[:, j, b, :]
                        for h in range(2):
                            nc.tensor.matmul(
                                ps[:, u * M + h * 512 : u * M + (h + 1) * 512],
                                lhsT,
                                rhs[:, b * M + h * 512 : b * M + (h + 1) * 512],
                            )
                    sbd = sbp.tile([P, 2 * M], bf16, tag="sbd")
                    nc.scalar.mul(sbd[:], ps[:], -1.0)  # +d2, bf16
                    for u in range(2):
                        col = b * NCHUNK + 2 * pr + u
                        nc.vector._custom_dve(
                            min_op,
                            out=junk[:],
                            in0=sbd[:, u * M : u * M + 512],
                            in1=sbd[:, u * M + 512 : (u + 1) * M],
                            s0=3.0e38,
                            accum_out=rowraw[:, col : col + 1],
                        )
                    if pr % 2 == 0:
                        pf = pfp.tile([P, M], bf16, tag="pf")
                        nc.vector.tensor_tensor(
                            out=pf[:], in0=sbd[:, 0:M], in1=sbd[:, M : 2 * M],
                            op=AL.min,
                        )
                        nc.sync.dma_start(oc2p[b, pr // 2], pf[:])
                    else:
                        nc.gpsimd.dma_start(oc2r[b, pr // 2], sbd[:])
                nc.sync.dma_start(
                    orow[:, b * NCHUNK : (b + 1) * NCHUNK],
                    rowraw[:, b * NCHUNK : (b + 1) * NCHUNK],
                )

    nc.compile()
    return nc


def _get_nc():
    if "nc" not in _CACHE:
        _CACHE["nc"] = _build_nc()
    return _CACHE["nc"]


def _split_bf16(x):
    import ml_dtypes

    hi = x.astype(ml_dtypes.bfloat16)
    lo = (x - hi.astype(np.float32)).astype(ml_dtypes.bfloat16)
    return hi, lo


def make_in_maps(skeletal_points, primitive_parameters, bspline_basis):
    import ml_dtypes

    bf16 = ml_dtypes.bfloat16
    skel = np.ascontiguousarray(skeletal_points, dtype=np.float32)
    prim = np.ascontiguousarray(primitive_parameters, dtype=np.float32)
    basis = np.ascontiguousarray(bspline_basis, dtype=np.float32)

    in_maps = []
    for c in range(NCORES):
        lh6 = np.empty((13, NCHUNK, NB, P), dtype=bf16)
        rhs = np.empty((13, NB * M), dtype=bf16)
        for b in range(NB):
            pts = skel[NB * c + b].reshape(P, JPP, 3)  # point p = 32r + j
            a_hi, a_lo = _split_bf16(pts)
            a2 = (pts.astype(np.float64) ** 2).sum(-1).astype(np.float32)
            a2_hi, a2_lo = _split_bf16(a2)
            for cc in range(3):
                lh6[0 + cc, :, b, :] = a_hi[:, :, cc].T
                lh6[3 + cc, :, b, :] = a_lo[:, :, cc].T
                lh6[6 + cc, :, b, :] = a_hi[:, :, cc].T
            lh6[9, :, b, :] = np.float32(1.0)
            lh6[10, :, b, :] = np.float32(1.0)
            lh6[11, :, b, :] = a2_hi.T
            lh6[12, :, b, :] = a2_lo.T

            ctrl = prim[NB * c + b].reshape(64, 4, 3)
            curves = np.einsum("tn,knc->ktc", basis, ctrl).reshape(M, 3)
            b2 = (curves.astype(np.float64) ** 2).sum(-1).astype(np.float32)
            r0, r1 = _split_bf16(2.0 * curves)
            nb2h, nb2l = _split_bf16(-b2)
            sl = slice(b * M, (b + 1) * M)
            for cc in range(3):
                rhs[0 + cc, sl] = r0[:, cc]
                rhs[3 + cc, sl] = r0[:, cc]
                rhs[6 + cc, sl] = r1[:, cc]
            rhs[9, sl] = nb2h
            rhs[10, sl] = nb2l
            rhs[11, sl] = np.float32(-1.0)
            rhs[12, sl] = np.float32(-1.0)
        in_maps.append({"lh6": lh6.reshape(13, NCHUNK * NB * P), "rhs": rhs})
    return in_maps


def postprocess(results):
    """results: list of 8 per-core dicts with orow/oc2."""
    import ml_dtypes

    loss = np.zeros(16, dtype=np.float32)
    for c, r in enumerate(results):
        rowd2 = np.maximum(np.asarray(r["orow"]).astype(np.float64), 0.0)  # (128, 64)
        # +d2 dumps. Fold as uint16: the bf16 bit pattern is monotonic for
        # values >= 0, and rare tiny negatives (psum rounding) sort above
        # everything, matching the relu semantics.
        ocp = (
            np.ascontiguousarray(np.asarray(r["oc2p"]))
            .view(np.uint16)
            .reshape(NB, NPAIR // 2, P, M)
        )
        ocr = (
            np.ascontiguousarray(np.asarray(r["oc2r"]))
            .view(np.uint16)
            .reshape(NB, NPAIR // 2, P, 2, M)
        )
        for b in range(NB):
            cha = np.sqrt(rowd2[:, b * NCHUNK : (b + 1) * NCHUNK]).mean()
            cu = np.minimum(
                ocp[b].min(axis=(0, 1)), ocr[b].min(axis=(0, 1, 2))
            )
            cold2 = cu.view(ml_dtypes.bfloat16).astype(np.float64)
            chb = np.sqrt(np.maximum(cold2, 0.0)).mean()
            loss[NB * c + b] = np.float32(cha + chb)
    return loss


def kernel(skeletal_points, primitive_parameters, bspline_basis):
    from concourse.bass_utils import run_bass_kernel_spmd

    nc = _get_nc()
    in_maps = make_in_maps(skeletal_points, primitive_parameters, bspline_basis)
    res = run_bass_kernel_spmd(nc, in_maps, core_ids=list(range(NCORES)))
    return postprocess(res.results)
